# revision 30
# baseline (speedup 1.0000x reference)
"""Trainium2 Bass kernel: LayerNorm -> MHA(16 heads, S=4096, D=1024) -> out-proj.

Sharding: tensor-parallel over heads. 8 cores x 2 heads each.
Each core computes LN(x) (replicated), q/k/v for its 2 heads (columns of
Wq/Wk/Wv), attention for those heads, and a partial output projection
(its 128 rows of Wo.T). Host sums the 8 partials and adds bo.

v2: all matmul operands fp16 (PE runs 16-bit at 2.4 GHz vs 1.2 GHz for
fp32r), out-projection folded into phase 2 per m-macro so its compute and
output DMA hide under the softmax-exp (ScalarE) shadow, fp16 output
partials (halves output DMA).

Per-core layout (core c, heads 2c, 2c+1; d-slice = [128c, 128c+128)):
  phase 1: LN in [m,d] tiles -> PE-transpose -> hT [d,m] (fp16)
           qT/kT = WT.T @ hT   [128n(2 heads' dims), 4096m] fp16
           v     = transpose(vT) -> [t, (tc,head,65)] fp16 (ones col for sums)
  phase 2: scoresT[t,m] = kT.T @ qT per head (K=64)  -> exp (ACT, scale=1/32)
           ctx[m,d]    += w.T @ v_aug (fp16, accumulates sums in col 64)
           normalize by 1/sums (per-partition), transpose -> ctxT [d,m] fp16
           then per-mac: partial out = ctxT.T @ WoT -> DRAM (fp16)

LN gain g is folded into Wq/Wk/Wv columns host-side; LN bias b_ln is folded
into bq/bk/bv.  bo is added host-side after the cross-core reduction.
"""

import math
from contextlib import ExitStack

import numpy as np

B, S, DIM, H = 1, 4096, 1024, 16
HD = DIM // H            # 64
N_CORES = 8
HPC = H // N_CORES       # 2 heads per core
DC = HPC * HD            # 128 dims per core
MB = 512                 # phase-1 m-block
N_MB = S // MB           # 8
TC = S // 128            # 32 t-chunks
MAC = 512                # phase-2 m-macro width
SCALE = 1.0 / math.sqrt(DIM)
QSCALE = math.log2(math.e) / math.sqrt(DIM)   # folded into Wq/bq host-side

_CACHE = {}
LAST_RESULT = None       # BassKernelResults of the most recent run (for test.py)

# exp(s/sqrt(DIM)) = 2^t with t = s*log2(e)/sqrt(DIM); the q-side weights are
# pre-scaled by log2(e)/sqrt(DIM) host-side so the scores matmul emits t
# directly.  ACT tiles compute 2^t = e^(t*ln2); DVE tiles evaluate a
# minimax deg-4 polynomial p(t) ~ 2^t on [-1.6, 1.6] (max rel err 1.0e-3)
# with p(0)=1 hardwired via the One constant: 8 ALU stages, exactly the
# custom-DVE budget.  Sharing the exp work between ScalarE and VectorE is
# what lets the Tensor engine stay the bottleneck (and stay HAM-warm).
EXP2_B = (0.69178671, 0.24190469, 0.05912525, 0.00915199)   # b1..b4
DVE_FRAC = 4             # every 4th (t,head) tile goes to the DVE
LN2 = math.log(2.0)


def _register_exp2_op():
    """Add the EXP2_P4_ANT custom-DVE op to the concourse registry.

    out = (((b4*t + b3)*t + b2)*t + b1)*t + 1  with b1..b3 via s0/s1/imm2
    and b4 spilled through in1 (C3 slot).  The uops_sha is computed here
    (self-consistent by construction) since the registry pins it."""
    import concourse.dve_ops as dom
    for o in dom.OPS:
        if o.name == "EXP2_P4_ANT":
            return o
    from concourse.dve_spec import (Spec, Src0, C0, C1, C2, C3, One,
                                    _spill_c3_to_src1, lower)
    from concourse.dve_uop import DveOpSpec

    body = _spill_c3_to_src1(
        ((((C3 * Src0) + C2) * Src0 + C1) * Src0 + C0) * Src0 + One)

    def _ref(in0, in1, c0, c1, c2):
        b4 = np.asarray(in1, np.float32).reshape(-1, 1)
        t = in0.astype(np.float32)
        return ((((b4 * t) + c2) * t + c1) * t + c0) * t + 1.0

    spec = Spec(body=body, reference=_ref)
    row = dom._CUSTOM_DVE_ROW_BASE + len(dom.OPS)
    shas = {}
    for ver in ("v3", "v4"):
        s = DveOpSpec(name="EXP2_P4_ANT", opcode=row,
                      uops=lower(spec, ver=ver), rd1_en=True)
        shas[ver] = s.sha(ver)
    op = dom.DveOp("EXP2_P4_ANT", spec, subdim=False, uops_sha=shas)
    dom.OPS.append(op)
    dom._SUB_OPCODE_FOR_NAME["EXP2_P4_ANT"] = row
    return op


def _build():
    import concourse.bacc as bacc
    import concourse.tile as tile
    import concourse.mybir as mybir
    from concourse.masks import make_identity

    dt = mybir.dt
    AF = mybir.ActivationFunctionType
    ALU = mybir.AluOpType

    exp2_op = _register_exp2_op()

    nc = bacc.Bacc("TRN2", target_bir_lowering=False, debug=False,
                   num_devices=N_CORES)

    x_d = nc.dram_tensor("x", [S, DIM], dt.float32, kind="ExternalInput")
    wqT_d = nc.dram_tensor("wqT", [DIM, DC], dt.float16, kind="ExternalInput")
    wkT_d = nc.dram_tensor("wkT", [DIM, DC], dt.float16, kind="ExternalInput")
    wvT_d = nc.dram_tensor("wvT", [DIM, DC], dt.float16, kind="ExternalInput")
    woT_d = nc.dram_tensor("woT", [DC, DIM], dt.float16, kind="ExternalInput")
    bq_d = nc.dram_tensor("bq", [DC], dt.float32, kind="ExternalInput")
    bk_d = nc.dram_tensor("bk", [DC], dt.float32, kind="ExternalInput")
    bv_d = nc.dram_tensor("bv", [DC], dt.float32, kind="ExternalInput")
    out_d = nc.dram_tensor("out", [S, DIM], dt.float16, kind="ExternalOutput")

    with tile.TileContext(nc) as tc, ExitStack() as top:
        persist = top.enter_context(tc.tile_pool(name="persist", bufs=1))

        # --- persistent tiles ---
        ident = persist.tile([128, 128], dt.float32)
        make_identity(nc, ident)
        ident16 = persist.tile([128, 128], dt.float16)
        nc.vector.tensor_copy(out=ident16, in_=ident)

        eps_t = persist.tile([128, 1], dt.float32)
        nc.vector.memset(eps_t, 1e-5)
        b4_t = persist.tile([128, 1], dt.float32)
        nc.vector.memset(b4_t, EXP2_B[3])

        wT = {}
        for name, d in (("q", wqT_d), ("k", wkT_d), ("v", wvT_d)):
            t = persist.tile([128, DIM // 128, DC], dt.float16,
                             tag=f"w{name}T", name=f"w{name}T")
            nc.sync.dma_start(out=t, in_=d.ap().rearrange(
                "(c p) n -> p c n", p=128))
            wT[name] = t
        woT = persist.tile([DC, DIM], dt.float16)
        nc.sync.dma_start(out=woT, in_=woT_d.ap())
        bias = {}
        for name, d in (("q", bq_d), ("k", bk_d), ("v", bv_d)):
            t = persist.tile([DC, 1], dt.float32, tag=f"b{name}",
                             name=f"b{name}")
            nc.sync.dma_start(out=t, in_=d.ap()[:, None])
            bias[name] = t

        qT_all = persist.tile([DC, S], dt.float16)
        kT_all = persist.tile([DC, S], dt.float16)
        # v with an appended ones-column per head: [t-part, tc, head, HD+1]
        v_all = persist.tile([128, TC, HPC, HD + 1], dt.float16)
        nc.vector.memset(v_all, 1.0)
        ctxT_all = persist.tile([DC, S], dt.float16)

        # ---------------- phase 1: LN + QKV projections ----------------
        with ExitStack() as p1:
            xpool = p1.enter_context(tc.tile_pool(name="xp", bufs=10))
            hpool = p1.enter_context(tc.tile_pool(name="hp", bufs=5))
            hTpool = p1.enter_context(tc.tile_pool(name="hTp", bufs=2))
            stat = p1.enter_context(tc.tile_pool(name="stat", bufs=8))
            vsb = p1.enter_context(tc.tile_pool(name="vsb", bufs=2))
            ps_t = p1.enter_context(tc.tile_pool(name="ps_t", bufs=2, space="PSUM"))
            ps_p = p1.enter_context(tc.tile_pool(name="ps_p", bufs=3, space="PSUM"))
            ps_v = p1.enter_context(tc.tile_pool(name="ps_v", bufs=1, space="PSUM"))

            for mb in range(N_MB):
                hs = []
                for j in range(MB // 128):
                    r0 = mb * MB + j * 128
                    xt = xpool.tile([128, DIM], dt.float32, tag="x")
                    nc.sync.dma_start(out=xt, in_=x_d.ap()[r0:r0 + 128, :])
                    # LayerNorm stats
                    st = stat.tile([128, 2, nc.vector.BN_STATS_DIM],
                                   dt.float32, tag="st")
                    xg = xt[:].rearrange("p (s f) -> p s f", s=2)
                    for sg in range(2):
                        nc.vector.bn_stats(out=st[:, sg, :], in_=xg[:, sg, :])
                    mv = stat.tile([128, 2], dt.float32, tag="mv")
                    nc.vector.bn_aggr(out=mv, in_=st)
                    std = stat.tile([128, 1], dt.float32, tag="sd")
                    nc.scalar.activation(out=std, in_=mv[:, 1:2], func=AF.Sqrt,
                                         bias=eps_t, scale=1.0)
                    rstd = stat.tile([128, 1], dt.float32, tag="rs")
                    nc.vector.reciprocal(out=rstd, in_=std)
                    ht = hpool.tile([128, DIM], dt.float16, tag="h")
                    nc.vector.tensor_scalar(out=ht, in0=xt, scalar1=mv[:, 0:1],
                                            scalar2=rstd, op0=ALU.subtract,
                                            op1=ALU.mult)
                    hs.append(ht)

                # transpose h -> hT  [128d, dc, 512m]
                hT = hTpool.tile([128, DIM // 128, MB], dt.float16, tag="hT")
                for dc in range(DIM // 128):
                    pt = ps_t.tile([128, MB], dt.float32, tag="pt")
                    for j in range(MB // 128):
                        nc.tensor.matmul(
                            pt[:, j * 128:(j + 1) * 128],
                            lhsT=hs[j][:, dc * 128:(dc + 1) * 128],
                            rhs=ident16, start=True, stop=True)
                    nc.scalar.copy(out=hT[:, dc, :], in_=pt)

                # q/k/v projections for this m-block: [128n, 512m]
                for name, dest in (("q", qT_all), ("k", kT_all), ("v", None)):
                    pp = ps_p.tile([128, MB], dt.float32, tag="pp")
                    for dc in range(DIM // 128):
                        nc.tensor.matmul(pp, lhsT=wT[name][:, dc, :],
                                         rhs=hT[:, dc, :],
                                         start=(dc == 0), stop=(dc == 7))
                    if dest is not None:
                        nc.scalar.activation(
                            out=dest[:, mb * MB:(mb + 1) * MB], in_=pp,
                            func=AF.Identity, bias=bias[name], scale=1.0)
                    else:
                        vT = vsb.tile([128, MB], dt.float16, tag="vT")
                        nc.vector.tensor_scalar(
                            out=vT, in0=pp, scalar1=bias[name], scalar2=None,
                            op0=ALU.add)
                        pv = ps_v.tile([128, MB], dt.float32, tag="pv")
                        for j in range(MB // 128):
                            nc.tensor.matmul(
                                pv[:, j * 128:(j + 1) * 128],
                                lhsT=vT[:, j * 128:(j + 1) * 128],
                                rhs=ident16, start=True, stop=True)
                        for j in range(MB // 128):
                            tc_j = mb * (MB // 128) + j
                            src = pv[:, j * 128:(j + 1) * 128].rearrange(
                                "p (h e) -> p h e", h=HPC)
                            nc.scalar.copy(
                                out=v_all[:, tc_j, :, 0:HD], in_=src)

        # ---------------- phase 2: attention + out-projection ----------------
        # ctxT-direct: lhsT = v_aug [128t, 65] (stationary), rhs = w [128t, m]
        # -> ctxT_u [65, m] accumulated in psum (row 64 = softmax sums).
        # Scores pipeline at [128,512] half-tile granularity (3 psum bufs);
        # exp halves are split ~60/40 between ScalarE and the custom
        # VectorE polynomial so neither engine gates the Tensor engine.
        # The previous mac's normalize + out-proj is INTERLEAVED into the
        # current mac's t-loop: keeps real matmuls flowing so the PE HAM
        # clock-gate never re-throttles on a transpose-only stretch.
        with ExitStack() as p2:
            sp0 = p2.enter_context(tc.tile_pool(name="sp0", bufs=2, space="PSUM"))
            sp1 = p2.enter_context(tc.tile_pool(name="sp1", bufs=2, space="PSUM"))
            spool = [sp0, sp1]
            cpool = p2.enter_context(tc.tile_pool(name="cp", bufs=2, space="PSUM"))
            fine = p2.enter_context(tc.tile_pool(name="fine", bufs=2, space="PSUM"))
            wpool = p2.enter_context(tc.tile_pool(name="wp", bufs=8))
            upool = p2.enter_context(tc.tile_pool(name="up", bufs=4))
            npool = p2.enter_context(tc.tile_pool(name="np", bufs=6))
            opool = p2.enter_context(tc.tile_pool(name="op", bufs=3))

            def norm_item(mac, ch, head, cu):
                hd0 = head * HD
                ptn = fine.tile([128, HD + 1], dt.float32, tag="f", name="ptn")
                nc.tensor.matmul(
                    ptn, lhsT=cu[:, ch * 128:(ch + 1) * 128],
                    rhs=ident16[0:HD + 1, 0:HD + 1], start=True, stop=True)
                rec = npool.tile([128, 1], dt.float32, tag="rec")
                nc.vector.reciprocal(out=rec, in_=ptn[:, HD:HD + 1])
                cn = npool.tile([128, HD], dt.float16, tag="cn")
                nc.vector.tensor_scalar(out=cn, in0=ptn[:, 0:HD],
                                        scalar1=rec, scalar2=None,
                                        op0=ALU.mult)
                ptx = fine.tile([HD, 128], dt.float32, tag="f", name="ptx")
                nc.tensor.matmul(ptx, lhsT=cn, rhs=ident16,
                                 start=True, stop=True)
                c0 = mac * MAC + ch * 128
                nc.vector.tensor_copy(
                    out=ctxT_all[hd0:hd0 + HD, c0:c0 + 128], in_=ptx)

            def oproj_item(mac, mc):
                c0 = mac * MAC + mc * 128
                ot = opool.tile([128, DIM], dt.float16, tag="o")
                for e in range(DIM // 512):
                    po = fine.tile([128, 512], dt.float32, tag="f", name="po")
                    nc.tensor.matmul(po, lhsT=ctxT_all[:, c0:c0 + 128],
                                     rhs=woT[:, e * 512:(e + 1) * 512],
                                     start=True, stop=True)
                    if e == 0:
                        nc.scalar.copy(out=ot[:, e * 512:(e + 1) * 512],
                                       in_=po)
                    else:
                        nc.vector.tensor_copy(
                            out=ot[:, e * 512:(e + 1) * 512], in_=po)
                nc.sync.dma_start(out=out_d.ap()[c0:c0 + 128, :], in_=ot)

            def finish_items(mac, cus):
                items = []
                for ch in range(MAC // 128):
                    for head in range(HPC):
                        items.append(
                            lambda m=mac, c=ch, h=head: norm_item(
                                m, c, h, cus[h]))
                    items.append(lambda m=mac, c=ch: oproj_item(m, c))
                return items

            pending = []
            for mac in range(S // MAC):
                m0 = mac * MAC
                pcu = [cpool.tile([HD + 1, MAC], dt.float32, tag="pc",
                                  name=f"pcu{i}") for i in range(HPC)]
                wq = []   # (t, head, w) exp outputs awaiting their ctx matmul
                for t in range(TC):
                    for head in range(HPC):
                        hd0 = head * HD
                        ps = spool[head].tile([128, MAC], dt.float32, tag="s")
                        # K=64: the two heads run on disjoint 64-row PE
                        # tiles concurrently.
                        nc.tensor.matmul(
                            ps,
                            lhsT=kT_all[hd0:hd0 + HD, t * 128:(t + 1) * 128],
                            rhs=qT_all[hd0:hd0 + HD, m0:m0 + MAC],
                            start=True, stop=True,
                            tile_position=(hd0, 0))
                        w = wpool.tile([128, MAC], dt.float16, tag="w")
                        # head0 -> ScalarE; head1 -> VectorE poly (1 in 8
                        # back on ScalarE to balance)
                        if head == 1 and t % 8 != 7:
                            nc.vector._custom_dve(
                                exp2_op, out=w[:], in0=ps[:], in1=b4_t[:],
                                s0=EXP2_B[0], s1=EXP2_B[1], imm2=EXP2_B[2])
                        else:
                            nc.scalar.activation(out=w, in_=ps,
                                                 func=AF.Exp, scale=LN2)
                        wq.append((t, head, w))
                    # issue ctx matmuls one t-iteration behind the exps so
                    # the PE never stalls on activation latency
                    while len(wq) > 2 * HPC:
                        wt, wh, ww = wq.pop(0)
                        nc.tensor.matmul(
                            pcu[wh], lhsT=v_all[:, wt, wh, :], rhs=ww,
                            start=(wt == 0), stop=(wt == TC - 1),
                            skip_group_check=True)
                    if pending and t % 3 != 0:
                        pending.pop(0)()
                for wt, wh, ww in wq:
                    nc.tensor.matmul(
                        pcu[wh], lhsT=v_all[:, wt, wh, :], rhs=ww,
                        start=(wt == 0), stop=(wt == TC - 1),
                        skip_group_check=True)
                # evacuate the accumulated ctx to SBUF (frees psum quickly);
                # normalize + out-proj run interleaved in the NEXT mac's loop
                for item in pending:   # leftovers (shouldn't happen)
                    item()
                cus = []
                for head in range(HPC):
                    cu = upool.tile([HD + 1, MAC], dt.float16, tag="cu")
                    nc.scalar.copy(out=cu, in_=pcu[head])
                    cus.append(cu)
                pending = finish_items(mac, cus)
            for item in pending:
                item()

    nc.compile()
    return nc


def kernel(**inputs):
    global LAST_RESULT
    from concourse.bass_utils import run_bass_kernel_spmd

    x = np.asarray(inputs["x"], dtype=np.float32).reshape(S, DIM)
    ln_g = np.asarray(inputs["ln_g"], dtype=np.float32)
    ln_b = np.asarray(inputs["ln_b"], dtype=np.float32)
    Wq = np.asarray(inputs["Wq"], dtype=np.float32)
    Wk = np.asarray(inputs["Wk"], dtype=np.float32)
    Wv = np.asarray(inputs["Wv"], dtype=np.float32)
    Wo = np.asarray(inputs["Wo"], dtype=np.float32)
    bq = np.asarray(inputs["bq"], dtype=np.float32)
    bk = np.asarray(inputs["bk"], dtype=np.float32)
    bv = np.asarray(inputs["bv"], dtype=np.float32)
    bo = np.asarray(inputs["bo"], dtype=np.float32)

    if "nc" not in _CACHE:
        _CACHE["nc"] = _build()
    nc = _CACHE["nc"]

    in_maps = []
    for c in range(N_CORES):
        sl = slice(c * DC, (c + 1) * DC)
        in_maps.append({
            "x": x,
            "wqT": np.ascontiguousarray(
                (Wq[sl] * ln_g[None, :]).T * QSCALE).astype(np.float16),
            "wkT": np.ascontiguousarray((Wk[sl] * ln_g[None, :]).T).astype(np.float16),
            "wvT": np.ascontiguousarray((Wv[sl] * ln_g[None, :]).T).astype(np.float16),
            "woT": np.ascontiguousarray(Wo[:, sl].T).astype(np.float16),
            "bq": (bq[sl] + Wq[sl] @ ln_b) * QSCALE,
            "bk": bk[sl] + Wk[sl] @ ln_b,
            "bv": bv[sl] + Wv[sl] @ ln_b,
        })

    res = run_bass_kernel_spmd(nc, in_maps, list(range(N_CORES)))
    LAST_RESULT = res

    acc = res.results[0]["out"].astype(np.float32)
    for c in range(1, N_CORES):
        acc = acc + res.results[c]["out"].astype(np.float32)
    acc += bo[None, :]
    return acc.reshape(B, S, DIM)


# revision 32
# speedup vs baseline: 1.0779x; 1.0779x over previous
"""Trainium2 Bass kernel: LayerNorm -> MHA(16 heads, S=4096, D=1024) -> out-proj.

Sharding: tensor-parallel over heads. 8 cores x 2 heads each.
Each core computes LN(x) (replicated), q/k/v for its 2 heads (columns of
Wq/Wk/Wv), attention for those heads, and a partial output projection
(its 128 rows of Wo.T). Host sums the 8 partials and adds bo.

v2: all matmul operands fp16 (PE runs 16-bit at 2.4 GHz vs 1.2 GHz for
fp32r), out-projection folded into phase 2 per m-macro so its compute and
output DMA hide under the softmax-exp (ScalarE) shadow, fp16 output
partials (halves output DMA).

Per-core layout (core c, heads 2c, 2c+1; d-slice = [128c, 128c+128)):
  phase 1: LN in [m,d] tiles -> PE-transpose -> hT [d,m] (fp16)
           qT/kT = WT.T @ hT   [128n(2 heads' dims), 4096m] fp16
           v     = transpose(vT) -> [t, (tc,head,65)] fp16 (ones col for sums)
  phase 2: scoresT[t,m] = kT.T @ qT per head (K=64)  -> exp (ACT, scale=1/32)
           ctx[m,d]    += w.T @ v_aug (fp16, accumulates sums in col 64)
           normalize by 1/sums (per-partition), transpose -> ctxT [d,m] fp16
           then per-mac: partial out = ctxT.T @ WoT -> DRAM (fp16)

LN gain g is folded into Wq/Wk/Wv columns host-side; LN bias b_ln is folded
into bq/bk/bv.  bo is added host-side after the cross-core reduction.
"""

import math
from contextlib import ExitStack

import numpy as np

B, S, DIM, H = 1, 4096, 1024, 16
HD = DIM // H            # 64
N_CORES = 8
HPC = H // N_CORES       # 2 heads per core
DC = HPC * HD            # 128 dims per core
MB = 512                 # phase-1 m-block
N_MB = S // MB           # 8
TC = S // 128            # 32 t-chunks
MAC = 512                # phase-2 m-macro width
SCALE = 1.0 / math.sqrt(DIM)
QSCALE = math.log2(math.e) / math.sqrt(DIM)   # folded into Wq/bq host-side

_CACHE = {}
LAST_RESULT = None       # BassKernelResults of the most recent run (for test.py)

# exp(s/sqrt(DIM)) = 2^t with t = s*log2(e)/sqrt(DIM); the q-side weights are
# pre-scaled by log2(e)/sqrt(DIM) host-side so the scores matmul emits t
# directly.  ACT tiles compute 2^t = e^(t*ln2); DVE tiles evaluate a
# minimax deg-4 polynomial p(t) ~ 2^t on [-1.6, 1.6] (max rel err 1.0e-3)
# with p(0)=1 hardwired via the One constant: 8 ALU stages, exactly the
# custom-DVE budget.  Sharing the exp work between ScalarE and VectorE is
# what lets the Tensor engine stay the bottleneck (and stay HAM-warm).
EXP2_B = (0.69178671, 0.24190469, 0.05912525, 0.00915199)   # b1..b4
DVE_FRAC = 4             # every 4th (t,head) tile goes to the DVE
LN2 = math.log(2.0)


def _register_exp2_op():
    """Add the EXP2_P4_ANT custom-DVE op to the concourse registry.

    out = (((b4*t + b3)*t + b2)*t + b1)*t + 1  with b1..b3 via s0/s1/imm2
    and b4 spilled through in1 (C3 slot).  The uops_sha is computed here
    (self-consistent by construction) since the registry pins it."""
    import concourse.dve_ops as dom
    for o in dom.OPS:
        if o.name == "EXP2_P4_ANT":
            return o
    from concourse.dve_spec import (Spec, Src0, C0, C1, C2, C3, One,
                                    _spill_c3_to_src1, lower)
    from concourse.dve_uop import DveOpSpec

    body = _spill_c3_to_src1(
        ((((C3 * Src0) + C2) * Src0 + C1) * Src0 + C0) * Src0 + One)

    def _ref(in0, in1, c0, c1, c2):
        b4 = np.asarray(in1, np.float32).reshape(-1, 1)
        t = in0.astype(np.float32)
        return ((((b4 * t) + c2) * t + c1) * t + c0) * t + 1.0

    spec = Spec(body=body, reference=_ref)
    row = dom._CUSTOM_DVE_ROW_BASE + len(dom.OPS)
    shas = {}
    for ver in ("v3", "v4"):
        s = DveOpSpec(name="EXP2_P4_ANT", opcode=row,
                      uops=lower(spec, ver=ver), rd1_en=True)
        shas[ver] = s.sha(ver)
    op = dom.DveOp("EXP2_P4_ANT", spec, subdim=False, uops_sha=shas)
    dom.OPS.append(op)
    dom._SUB_OPCODE_FOR_NAME["EXP2_P4_ANT"] = row
    return op


def _build():
    import concourse.bacc as bacc
    import concourse.tile as tile
    import concourse.mybir as mybir
    from concourse.masks import make_identity

    dt = mybir.dt
    AF = mybir.ActivationFunctionType
    ALU = mybir.AluOpType

    exp2_op = _register_exp2_op()

    nc = bacc.Bacc("TRN2", target_bir_lowering=False, debug=False,
                   num_devices=N_CORES)

    x_d = nc.dram_tensor("x", [S, DIM], dt.float32, kind="ExternalInput")
    wqT_d = nc.dram_tensor("wqT", [DIM, DC], dt.float16, kind="ExternalInput")
    wkT_d = nc.dram_tensor("wkT", [DIM, DC], dt.float16, kind="ExternalInput")
    wvT_d = nc.dram_tensor("wvT", [DIM, DC], dt.float16, kind="ExternalInput")
    woT_d = nc.dram_tensor("woT", [DC, DIM], dt.float16, kind="ExternalInput")
    bq_d = nc.dram_tensor("bq", [DC], dt.float32, kind="ExternalInput")
    bk_d = nc.dram_tensor("bk", [DC], dt.float32, kind="ExternalInput")
    bv_d = nc.dram_tensor("bv", [DC], dt.float32, kind="ExternalInput")
    out_d = nc.dram_tensor("out", [S, DIM], dt.float16, kind="ExternalOutput")

    with tile.TileContext(nc) as tc, ExitStack() as top:
        persist = top.enter_context(tc.tile_pool(name="persist", bufs=1))

        # --- persistent tiles ---
        ident = persist.tile([128, 128], dt.float32)
        make_identity(nc, ident)
        ident16 = persist.tile([128, 128], dt.float16)
        nc.vector.tensor_copy(out=ident16, in_=ident)

        eps_t = persist.tile([128, 1], dt.float32)
        nc.vector.memset(eps_t, 1e-5)
        b4_t = persist.tile([128, 1], dt.float32)
        nc.vector.memset(b4_t, EXP2_B[3])

        wT = {}
        for name, d in (("q", wqT_d), ("k", wkT_d), ("v", wvT_d)):
            t = persist.tile([128, DIM // 128, DC], dt.float16,
                             tag=f"w{name}T", name=f"w{name}T")
            nc.sync.dma_start(out=t, in_=d.ap().rearrange(
                "(c p) n -> p c n", p=128))
            wT[name] = t
        woT = persist.tile([DC, DIM], dt.float16)
        nc.sync.dma_start(out=woT, in_=woT_d.ap())
        bias = {}
        for name, d in (("q", bq_d), ("k", bk_d), ("v", bv_d)):
            t = persist.tile([DC, 1], dt.float32, tag=f"b{name}",
                             name=f"b{name}")
            nc.sync.dma_start(out=t, in_=d.ap()[:, None])
            bias[name] = t

        qT_all = persist.tile([DC, S], dt.float16)
        kT_all = persist.tile([DC, S], dt.float16)
        # v with an appended ones-column per head: [t-part, tc, head, HD+1]
        v_all = persist.tile([128, TC, HPC, HD + 1], dt.float16)
        nc.vector.memset(v_all, 1.0)
        ctxT_all = persist.tile([DC, S], dt.float16)

        # ---------------- phase 1: LN + QKV projections ----------------
        with ExitStack() as p1:
            xpool = p1.enter_context(tc.tile_pool(name="xp", bufs=10))
            hpool = p1.enter_context(tc.tile_pool(name="hp", bufs=5))
            hTpool = p1.enter_context(tc.tile_pool(name="hTp", bufs=2))
            stat = p1.enter_context(tc.tile_pool(name="stat", bufs=8))
            vsb = p1.enter_context(tc.tile_pool(name="vsb", bufs=2))
            ps_t = p1.enter_context(tc.tile_pool(name="ps_t", bufs=2, space="PSUM"))
            ps_p = p1.enter_context(tc.tile_pool(name="ps_p", bufs=3, space="PSUM"))
            ps_v = p1.enter_context(tc.tile_pool(name="ps_v", bufs=1, space="PSUM"))

            for mb in range(N_MB):
                hs = []
                for j in range(MB // 128):
                    r0 = mb * MB + j * 128
                    xt = xpool.tile([128, DIM], dt.float32, tag="x")
                    nc.sync.dma_start(out=xt, in_=x_d.ap()[r0:r0 + 128, :])
                    # LayerNorm stats
                    st = stat.tile([128, 2, nc.vector.BN_STATS_DIM],
                                   dt.float32, tag="st")
                    xg = xt[:].rearrange("p (s f) -> p s f", s=2)
                    for sg in range(2):
                        nc.vector.bn_stats(out=st[:, sg, :], in_=xg[:, sg, :])
                    mv = stat.tile([128, 2], dt.float32, tag="mv")
                    nc.vector.bn_aggr(out=mv, in_=st)
                    std = stat.tile([128, 1], dt.float32, tag="sd")
                    nc.scalar.activation(out=std, in_=mv[:, 1:2], func=AF.Sqrt,
                                         bias=eps_t, scale=1.0)
                    rstd = stat.tile([128, 1], dt.float32, tag="rs")
                    nc.vector.reciprocal(out=rstd, in_=std)
                    ht = hpool.tile([128, DIM], dt.float16, tag="h")
                    nc.vector.tensor_scalar(out=ht, in0=xt, scalar1=mv[:, 0:1],
                                            scalar2=rstd, op0=ALU.subtract,
                                            op1=ALU.mult)
                    hs.append(ht)

                # transpose h -> hT  [128d, dc, 512m]
                hT = hTpool.tile([128, DIM // 128, MB], dt.float16, tag="hT")
                for dc in range(DIM // 128):
                    pt = ps_t.tile([128, MB], dt.float32, tag="pt")
                    for j in range(MB // 128):
                        nc.tensor.matmul(
                            pt[:, j * 128:(j + 1) * 128],
                            lhsT=hs[j][:, dc * 128:(dc + 1) * 128],
                            rhs=ident16, start=True, stop=True)
                    nc.scalar.copy(out=hT[:, dc, :], in_=pt)

                # q/k/v projections for this m-block: [128n, 512m]
                for name, dest in (("q", qT_all), ("k", kT_all), ("v", None)):
                    pp = ps_p.tile([128, MB], dt.float32, tag="pp")
                    for dc in range(DIM // 128):
                        nc.tensor.matmul(pp, lhsT=wT[name][:, dc, :],
                                         rhs=hT[:, dc, :],
                                         start=(dc == 0), stop=(dc == 7))
                    if dest is not None:
                        nc.scalar.activation(
                            out=dest[:, mb * MB:(mb + 1) * MB], in_=pp,
                            func=AF.Identity, bias=bias[name], scale=1.0)
                    else:
                        vT = vsb.tile([128, MB], dt.float16, tag="vT")
                        nc.vector.tensor_scalar(
                            out=vT, in0=pp, scalar1=bias[name], scalar2=None,
                            op0=ALU.add)
                        pv = ps_v.tile([128, MB], dt.float32, tag="pv")
                        for j in range(MB // 128):
                            nc.tensor.matmul(
                                pv[:, j * 128:(j + 1) * 128],
                                lhsT=vT[:, j * 128:(j + 1) * 128],
                                rhs=ident16, start=True, stop=True)
                        for j in range(MB // 128):
                            tc_j = mb * (MB // 128) + j
                            src = pv[:, j * 128:(j + 1) * 128].rearrange(
                                "p (h e) -> p h e", h=HPC)
                            nc.scalar.copy(
                                out=v_all[:, tc_j, :, 0:HD], in_=src)

        # ---------------- phase 2: attention + out-projection ----------------
        # ctxT-direct: lhsT = v_aug [128t, 65] (stationary), rhs = w [128t, m]
        # -> ctxT_u [65, m] accumulated in psum (row 64 = softmax sums).
        # Scores pipeline at [128,512] half-tile granularity (3 psum bufs);
        # exp halves are split ~60/40 between ScalarE and the custom
        # VectorE polynomial so neither engine gates the Tensor engine.
        # The previous mac's normalize + out-proj is INTERLEAVED into the
        # current mac's t-loop: keeps real matmuls flowing so the PE HAM
        # clock-gate never re-throttles on a transpose-only stretch.
        with ExitStack() as p2:
            sp0 = p2.enter_context(tc.tile_pool(name="sp0", bufs=2, space="PSUM"))
            sp1 = p2.enter_context(tc.tile_pool(name="sp1", bufs=2, space="PSUM"))
            spool = [sp0, sp1]
            cpool = p2.enter_context(tc.tile_pool(name="cp", bufs=2, space="PSUM"))
            fine = p2.enter_context(tc.tile_pool(name="fine", bufs=2, space="PSUM"))
            wpool = p2.enter_context(tc.tile_pool(name="wp", bufs=8))
            upool = p2.enter_context(tc.tile_pool(name="up", bufs=4))
            npool = p2.enter_context(tc.tile_pool(name="np", bufs=6))
            opool = p2.enter_context(tc.tile_pool(name="op", bufs=3))

            NCH = MAC // 128

            def norm_stage1(head, cu, state):
                # transpose the whole mac's ctx+sums for one head, batch the
                # reciprocal and the 1/sums multiply (stride 66 keeps each
                # matmul's psum write 8-byte aligned)
                ptn4 = fine.tile([128, NCH, HD + 2], dt.float32, tag="f",
                                 name="ptn4")
                for ch in range(NCH):
                    nc.tensor.matmul(
                        ptn4[:, ch, 0:HD + 1],
                        lhsT=cu[:, ch * 128:(ch + 1) * 128],
                        rhs=ident16[0:HD + 1, 0:HD + 1],
                        start=True, stop=True)
                rec4 = npool.tile([128, NCH], dt.float32, tag="rec")
                nc.vector.reciprocal(out=rec4, in_=ptn4[:, :, HD])
                cn4 = npool.tile([128, NCH, HD], dt.float16, tag="cn")
                nc.vector.tensor_tensor(
                    out=cn4, in0=ptn4[:, :, 0:HD],
                    in1=rec4[:].unsqueeze(-1).broadcast_to([128, NCH, HD]),
                    op=ALU.mult)
                state[head] = cn4

            def norm_stage2(mac, ch, state):
                c0 = mac * MAC + ch * 128
                for head in range(HPC):
                    hd0 = head * HD
                    ptx = fine.tile([HD, 128], dt.float32, tag="f", name="ptx")
                    nc.tensor.matmul(ptx, lhsT=state[head][:, ch, :],
                                     rhs=ident16, start=True, stop=True)
                    nc.vector.tensor_copy(
                        out=ctxT_all[hd0:hd0 + HD, c0:c0 + 128], in_=ptx)

            def oproj_item(mac, mc):
                c0 = mac * MAC + mc * 128
                ot = opool.tile([128, DIM], dt.float16, tag="o")
                for e in range(DIM // 512):
                    po = fine.tile([128, 512], dt.float32, tag="f", name="po")
                    nc.tensor.matmul(po, lhsT=ctxT_all[:, c0:c0 + 128],
                                     rhs=woT[:, e * 512:(e + 1) * 512],
                                     start=True, stop=True)
                    if e == 0:
                        nc.scalar.copy(out=ot[:, e * 512:(e + 1) * 512],
                                       in_=po)
                    else:
                        nc.vector.tensor_copy(
                            out=ot[:, e * 512:(e + 1) * 512], in_=po)
                nc.sync.dma_start(out=out_d.ap()[c0:c0 + 128, :], in_=ot)

            def finish_items(mac, cus):
                state = {}
                items = []
                for head in range(HPC):
                    items.append(
                        lambda h=head: norm_stage1(h, cus[h], state))
                for ch in range(MAC // 128):
                    items.append(lambda m=mac, c=ch: norm_stage2(m, c, state))
                    items.append(lambda m=mac, c=ch: oproj_item(m, c))
                return items

            pending = []
            for mac in range(S // MAC):
                m0 = mac * MAC
                pcu = [cpool.tile([HD + 1, MAC], dt.float32, tag="pc",
                                  name=f"pcu{i}") for i in range(HPC)]
                wq = []   # (t, head, w) exp outputs awaiting their ctx matmul
                for t in range(TC):
                    for head in range(HPC):
                        hd0 = head * HD
                        ps = spool[head].tile([128, MAC], dt.float32, tag="s")
                        # K=64: the two heads run on disjoint 64-row PE
                        # tiles concurrently.
                        nc.tensor.matmul(
                            ps,
                            lhsT=kT_all[hd0:hd0 + HD, t * 128:(t + 1) * 128],
                            rhs=qT_all[hd0:hd0 + HD, m0:m0 + MAC],
                            start=True, stop=True,
                            tile_position=(hd0, 0))
                        w = wpool.tile([128, MAC], dt.float16, tag="w")
                        # head0 -> ScalarE; head1 -> VectorE poly (1 in 8
                        # back on ScalarE to balance)
                        if head == 1 and t % 8 != 7:
                            nc.vector._custom_dve(
                                exp2_op, out=w[:], in0=ps[:], in1=b4_t[:],
                                s0=EXP2_B[0], s1=EXP2_B[1], imm2=EXP2_B[2])
                        else:
                            nc.scalar.activation(out=w, in_=ps,
                                                 func=AF.Exp, scale=LN2)
                        wq.append((t, head, w))
                    # issue ctx matmuls one t-iteration behind the exps so
                    # the PE never stalls on activation latency
                    while len(wq) > 2 * HPC:
                        wt, wh, ww = wq.pop(0)
                        nc.tensor.matmul(
                            pcu[wh], lhsT=v_all[:, wt, wh, :], rhs=ww,
                            start=(wt == 0), stop=(wt == TC - 1),
                            skip_group_check=True)
                    if pending and t % 3 != 0:
                        pending.pop(0)()
                for wt, wh, ww in wq:
                    nc.tensor.matmul(
                        pcu[wh], lhsT=v_all[:, wt, wh, :], rhs=ww,
                        start=(wt == 0), stop=(wt == TC - 1),
                        skip_group_check=True)
                # evacuate the accumulated ctx to SBUF (frees psum quickly);
                # normalize + out-proj run interleaved in the NEXT mac's loop
                for item in pending:   # leftovers (shouldn't happen)
                    item()
                cus = []
                for head in range(HPC):
                    cu = upool.tile([HD + 1, MAC], dt.float16, tag="cu")
                    nc.scalar.copy(out=cu, in_=pcu[head])
                    cus.append(cu)
                pending = finish_items(mac, cus)
            for item in pending:
                item()

    nc.compile()
    return nc


def kernel(**inputs):
    global LAST_RESULT
    from concourse.bass_utils import run_bass_kernel_spmd

    x = np.asarray(inputs["x"], dtype=np.float32).reshape(S, DIM)
    ln_g = np.asarray(inputs["ln_g"], dtype=np.float32)
    ln_b = np.asarray(inputs["ln_b"], dtype=np.float32)
    Wq = np.asarray(inputs["Wq"], dtype=np.float32)
    Wk = np.asarray(inputs["Wk"], dtype=np.float32)
    Wv = np.asarray(inputs["Wv"], dtype=np.float32)
    Wo = np.asarray(inputs["Wo"], dtype=np.float32)
    bq = np.asarray(inputs["bq"], dtype=np.float32)
    bk = np.asarray(inputs["bk"], dtype=np.float32)
    bv = np.asarray(inputs["bv"], dtype=np.float32)
    bo = np.asarray(inputs["bo"], dtype=np.float32)

    if "nc" not in _CACHE:
        _CACHE["nc"] = _build()
    nc = _CACHE["nc"]

    in_maps = []
    for c in range(N_CORES):
        sl = slice(c * DC, (c + 1) * DC)
        in_maps.append({
            "x": x,
            "wqT": np.ascontiguousarray(
                (Wq[sl] * ln_g[None, :]).T * QSCALE).astype(np.float16),
            "wkT": np.ascontiguousarray((Wk[sl] * ln_g[None, :]).T).astype(np.float16),
            "wvT": np.ascontiguousarray((Wv[sl] * ln_g[None, :]).T).astype(np.float16),
            "woT": np.ascontiguousarray(Wo[:, sl].T).astype(np.float16),
            "bq": (bq[sl] + Wq[sl] @ ln_b) * QSCALE,
            "bk": bk[sl] + Wk[sl] @ ln_b,
            "bv": bv[sl] + Wv[sl] @ ln_b,
        })

    res = run_bass_kernel_spmd(nc, in_maps, list(range(N_CORES)))
    LAST_RESULT = res

    acc = res.results[0]["out"].astype(np.float32)
    for c in range(1, N_CORES):
        acc = acc + res.results[c]["out"].astype(np.float32)
    acc += bo[None, :]
    return acc.reshape(B, S, DIM)


# revision 33
# speedup vs baseline: 1.1048x; 1.0249x over previous
"""Trainium2 Bass kernel: LayerNorm -> MHA(16 heads, S=4096, D=1024) -> out-proj.

Sharding: tensor-parallel over heads. 8 cores x 2 heads each.
Each core computes LN(x) (replicated), q/k/v for its 2 heads (columns of
Wq/Wk/Wv), attention for those heads, and a partial output projection
(its 128 rows of Wo.T). Host sums the 8 partials and adds bo.

v2: all matmul operands fp16 (PE runs 16-bit at 2.4 GHz vs 1.2 GHz for
fp32r), out-projection folded into phase 2 per m-macro so its compute and
output DMA hide under the softmax-exp (ScalarE) shadow, fp16 output
partials (halves output DMA).

Per-core layout (core c, heads 2c, 2c+1; d-slice = [128c, 128c+128)):
  phase 1: LN in [m,d] tiles -> PE-transpose -> hT [d,m] (fp16)
           qT/kT = WT.T @ hT   [128n(2 heads' dims), 4096m] fp16
           v     = transpose(vT) -> [t, (tc,head,65)] fp16 (ones col for sums)
  phase 2: scoresT[t,m] = kT.T @ qT per head (K=64)  -> exp (ACT, scale=1/32)
           ctx[m,d]    += w.T @ v_aug (fp16, accumulates sums in col 64)
           normalize by 1/sums (per-partition), transpose -> ctxT [d,m] fp16
           then per-mac: partial out = ctxT.T @ WoT -> DRAM (fp16)

LN gain g is folded into Wq/Wk/Wv columns host-side; LN bias b_ln is folded
into bq/bk/bv.  bo is added host-side after the cross-core reduction.
"""

import math
from contextlib import ExitStack

import numpy as np

B, S, DIM, H = 1, 4096, 1024, 16
HD = DIM // H            # 64
N_CORES = 8
HPC = H // N_CORES       # 2 heads per core
DC = HPC * HD            # 128 dims per core
MB = 512                 # phase-1 m-block
N_MB = S // MB           # 8
TC = S // 128            # 32 t-chunks
MAC = 512                # phase-2 m-macro width
SCALE = 1.0 / math.sqrt(DIM)
QSCALE = math.log2(math.e) / math.sqrt(DIM)   # folded into Wq/bq host-side

_CACHE = {}
LAST_RESULT = None       # BassKernelResults of the most recent run (for test.py)

# exp(s/sqrt(DIM)) = 2^t with t = s*log2(e)/sqrt(DIM); the q-side weights are
# pre-scaled by log2(e)/sqrt(DIM) host-side so the scores matmul emits t
# directly.  ACT tiles compute 2^t = e^(t*ln2); DVE tiles evaluate a
# minimax deg-4 polynomial p(t) ~ 2^t on [-1.6, 1.6] (max rel err 1.0e-3)
# with p(0)=1 hardwired via the One constant: 8 ALU stages, exactly the
# custom-DVE budget.  Sharing the exp work between ScalarE and VectorE is
# what lets the Tensor engine stay the bottleneck (and stay HAM-warm).
EXP2_B = (0.69178671, 0.24190469, 0.05912525, 0.00915199)   # b1..b4
DVE_FRAC = 4             # every 4th (t,head) tile goes to the DVE
LN2 = math.log(2.0)


def _register_exp2_op():
    """Add the EXP2_P4_ANT custom-DVE op to the concourse registry.

    out = (((b4*t + b3)*t + b2)*t + b1)*t + 1  with b1..b3 via s0/s1/imm2
    and b4 spilled through in1 (C3 slot).  The uops_sha is computed here
    (self-consistent by construction) since the registry pins it."""
    import concourse.dve_ops as dom
    for o in dom.OPS:
        if o.name == "EXP2_P4_ANT":
            return o
    from concourse.dve_spec import (Spec, Src0, C0, C1, C2, C3, One,
                                    _spill_c3_to_src1, lower)
    from concourse.dve_uop import DveOpSpec

    body = _spill_c3_to_src1(
        ((((C3 * Src0) + C2) * Src0 + C1) * Src0 + C0) * Src0 + One)

    def _ref(in0, in1, c0, c1, c2):
        b4 = np.asarray(in1, np.float32).reshape(-1, 1)
        t = in0.astype(np.float32)
        return ((((b4 * t) + c2) * t + c1) * t + c0) * t + 1.0

    spec = Spec(body=body, reference=_ref)
    row = dom._CUSTOM_DVE_ROW_BASE + len(dom.OPS)
    shas = {}
    for ver in ("v3", "v4"):
        s = DveOpSpec(name="EXP2_P4_ANT", opcode=row,
                      uops=lower(spec, ver=ver), rd1_en=True)
        shas[ver] = s.sha(ver)
    op = dom.DveOp("EXP2_P4_ANT", spec, subdim=False, uops_sha=shas)
    dom.OPS.append(op)
    dom._SUB_OPCODE_FOR_NAME["EXP2_P4_ANT"] = row
    return op


def _build():
    import concourse.bacc as bacc
    import concourse.tile as tile
    import concourse.mybir as mybir
    from concourse.masks import make_identity

    dt = mybir.dt
    AF = mybir.ActivationFunctionType
    ALU = mybir.AluOpType

    exp2_op = _register_exp2_op()

    nc = bacc.Bacc("TRN2", target_bir_lowering=False, debug=False,
                   num_devices=N_CORES)

    x_d = nc.dram_tensor("x", [S, DIM], dt.float32, kind="ExternalInput")
    wqT_d = nc.dram_tensor("wqT", [DIM, DC], dt.float16, kind="ExternalInput")
    wkT_d = nc.dram_tensor("wkT", [DIM, DC], dt.float16, kind="ExternalInput")
    wvT_d = nc.dram_tensor("wvT", [DIM, DC], dt.float16, kind="ExternalInput")
    woT_d = nc.dram_tensor("woT", [DC, DIM], dt.float16, kind="ExternalInput")
    bq_d = nc.dram_tensor("bq", [DC], dt.float32, kind="ExternalInput")
    bk_d = nc.dram_tensor("bk", [DC], dt.float32, kind="ExternalInput")
    bv_d = nc.dram_tensor("bv", [DC], dt.float32, kind="ExternalInput")
    out_d = nc.dram_tensor("out", [S, DIM], dt.float16, kind="ExternalOutput")

    with tile.TileContext(nc) as tc, ExitStack() as top:
        persist = top.enter_context(tc.tile_pool(name="persist", bufs=1))

        # --- persistent tiles ---
        ident = persist.tile([128, 128], dt.float32)
        make_identity(nc, ident)
        ident16 = persist.tile([128, 128], dt.float16)
        nc.vector.tensor_copy(out=ident16, in_=ident)

        eps_t = persist.tile([128, 1], dt.float32)
        nc.vector.memset(eps_t, 1e-5)
        b4_t = persist.tile([128, 1], dt.float32)
        nc.vector.memset(b4_t, EXP2_B[3])

        wT = {}
        for name, d in (("q", wqT_d), ("k", wkT_d), ("v", wvT_d)):
            t = persist.tile([128, DIM // 128, DC], dt.float16,
                             tag=f"w{name}T", name=f"w{name}T")
            nc.sync.dma_start(out=t, in_=d.ap().rearrange(
                "(c p) n -> p c n", p=128))
            wT[name] = t
        woT = persist.tile([DC, DIM], dt.float16)
        nc.sync.dma_start(out=woT, in_=woT_d.ap())
        bias = {}
        for name, d in (("q", bq_d), ("k", bk_d), ("v", bv_d)):
            t = persist.tile([DC, 1], dt.float32, tag=f"b{name}",
                             name=f"b{name}")
            nc.sync.dma_start(out=t, in_=d.ap()[:, None])
            bias[name] = t

        qT_all = persist.tile([DC, S], dt.float16)
        kT_all = persist.tile([DC, S], dt.float16)
        # v with an appended ones-column per head: [t-part, tc, head, HD+1]
        v_all = persist.tile([128, TC, HPC, HD + 1], dt.float16)
        nc.vector.memset(v_all, 1.0)
        ctxT_all = persist.tile([DC, S], dt.float16)

        # ---------------- phase 1: LN + QKV projections ----------------
        with ExitStack() as p1:
            xpool = p1.enter_context(tc.tile_pool(name="xp", bufs=10))
            hpool = p1.enter_context(tc.tile_pool(name="hp", bufs=5))
            hTpool = p1.enter_context(tc.tile_pool(name="hTp", bufs=2))
            stat = p1.enter_context(tc.tile_pool(name="stat", bufs=8))
            vsb = p1.enter_context(tc.tile_pool(name="vsb", bufs=2))
            ps_t = p1.enter_context(tc.tile_pool(name="ps_t", bufs=2, space="PSUM"))
            ps_p = p1.enter_context(tc.tile_pool(name="ps_p", bufs=3, space="PSUM"))
            ps_v = p1.enter_context(tc.tile_pool(name="ps_v", bufs=1, space="PSUM"))

            for mb in range(N_MB):
                hs = []
                for j in range(MB // 128):
                    r0 = mb * MB + j * 128
                    xt = xpool.tile([128, DIM], dt.float32, tag="x")
                    nc.sync.dma_start(out=xt, in_=x_d.ap()[r0:r0 + 128, :])
                    # LayerNorm stats
                    st = stat.tile([128, 2, nc.vector.BN_STATS_DIM],
                                   dt.float32, tag="st")
                    xg = xt[:].rearrange("p (s f) -> p s f", s=2)
                    for sg in range(2):
                        nc.vector.bn_stats(out=st[:, sg, :], in_=xg[:, sg, :])
                    mv = stat.tile([128, 2], dt.float32, tag="mv")
                    nc.vector.bn_aggr(out=mv, in_=st)
                    std = stat.tile([128, 1], dt.float32, tag="sd")
                    nc.scalar.activation(out=std, in_=mv[:, 1:2], func=AF.Sqrt,
                                         bias=eps_t, scale=1.0)
                    rstd = stat.tile([128, 1], dt.float32, tag="rs")
                    nc.vector.reciprocal(out=rstd, in_=std)
                    ht = hpool.tile([128, DIM], dt.float16, tag="h")
                    nc.vector.tensor_scalar(out=ht, in0=xt, scalar1=mv[:, 0:1],
                                            scalar2=rstd, op0=ALU.subtract,
                                            op1=ALU.mult)
                    hs.append(ht)

                # transpose h -> hT  [128d, dc, 512m]
                hT = hTpool.tile([128, DIM // 128, MB], dt.float16, tag="hT")
                for dc in range(DIM // 128):
                    pt = ps_t.tile([128, MB], dt.float32, tag="pt")
                    for j in range(MB // 128):
                        nc.tensor.matmul(
                            pt[:, j * 128:(j + 1) * 128],
                            lhsT=hs[j][:, dc * 128:(dc + 1) * 128],
                            rhs=ident16, start=True, stop=True)
                    nc.scalar.copy(out=hT[:, dc, :], in_=pt)

                # q/k/v projections for this m-block: [128n, 512m]
                for name, dest in (("q", qT_all), ("k", kT_all), ("v", None)):
                    pp = ps_p.tile([128, MB], dt.float32, tag="pp")
                    for dc in range(DIM // 128):
                        nc.tensor.matmul(pp, lhsT=wT[name][:, dc, :],
                                         rhs=hT[:, dc, :],
                                         start=(dc == 0), stop=(dc == 7))
                    if dest is not None:
                        nc.scalar.activation(
                            out=dest[:, mb * MB:(mb + 1) * MB], in_=pp,
                            func=AF.Identity, bias=bias[name], scale=1.0)
                    else:
                        vT = vsb.tile([128, MB], dt.float16, tag="vT")
                        nc.vector.tensor_scalar(
                            out=vT, in0=pp, scalar1=bias[name], scalar2=None,
                            op0=ALU.add)
                        pv = ps_v.tile([128, MB], dt.float32, tag="pv")
                        for j in range(MB // 128):
                            nc.tensor.matmul(
                                pv[:, j * 128:(j + 1) * 128],
                                lhsT=vT[:, j * 128:(j + 1) * 128],
                                rhs=ident16, start=True, stop=True)
                        for j in range(MB // 128):
                            tc_j = mb * (MB // 128) + j
                            src = pv[:, j * 128:(j + 1) * 128].rearrange(
                                "p (h e) -> p h e", h=HPC)
                            nc.scalar.copy(
                                out=v_all[:, tc_j, :, 0:HD], in_=src)

        # ---------------- phase 2: attention + out-projection ----------------
        # ctxT-direct: lhsT = v_aug [128t, 65] (stationary), rhs = w [128t, m]
        # -> ctxT_u [65, m] accumulated in psum (row 64 = softmax sums).
        # Scores pipeline at [128,512] half-tile granularity (3 psum bufs);
        # exp halves are split ~60/40 between ScalarE and the custom
        # VectorE polynomial so neither engine gates the Tensor engine.
        # The previous mac's normalize + out-proj is INTERLEAVED into the
        # current mac's t-loop: keeps real matmuls flowing so the PE HAM
        # clock-gate never re-throttles on a transpose-only stretch.
        with ExitStack() as p2:
            sp0 = p2.enter_context(tc.tile_pool(name="sp0", bufs=2, space="PSUM"))
            sp1 = p2.enter_context(tc.tile_pool(name="sp1", bufs=2, space="PSUM"))
            spool = [sp0, sp1]
            cpool = p2.enter_context(tc.tile_pool(name="cp", bufs=2, space="PSUM"))
            fine = p2.enter_context(tc.tile_pool(name="fine", bufs=2, space="PSUM"))
            wpool = p2.enter_context(tc.tile_pool(name="wp", bufs=8))
            upool = p2.enter_context(tc.tile_pool(name="up", bufs=4))
            npool = p2.enter_context(tc.tile_pool(name="np", bufs=6))
            opool = p2.enter_context(tc.tile_pool(name="op", bufs=3))

            NCH = MAC // 128

            def norm_stage1(head, cu, state):
                # transpose the whole mac's ctx+sums for one head, batch the
                # reciprocal and the 1/sums multiply (stride 66 keeps each
                # matmul's psum write 8-byte aligned)
                ptn4 = fine.tile([128, NCH, HD + 2], dt.float32, tag="f",
                                 name="ptn4")
                for ch in range(NCH):
                    nc.tensor.matmul(
                        ptn4[:, ch, 0:HD + 1],
                        lhsT=cu[:, ch * 128:(ch + 1) * 128],
                        rhs=ident16[0:HD + 1, 0:HD + 1],
                        start=True, stop=True)
                rec4 = npool.tile([128, NCH], dt.float32, tag="rec")
                nc.vector.reciprocal(out=rec4, in_=ptn4[:, :, HD])
                cn4 = npool.tile([128, NCH, HD], dt.float16, tag="cn")
                nc.vector.tensor_tensor(
                    out=cn4, in0=ptn4[:, :, 0:HD],
                    in1=rec4[:].unsqueeze(-1).broadcast_to([128, NCH, HD]),
                    op=ALU.mult)
                state[head] = cn4

            def norm_stage2(mac, ch, state):
                c0 = mac * MAC + ch * 128
                for head in range(HPC):
                    hd0 = head * HD
                    ptx = fine.tile([HD, 128], dt.float32, tag="f", name="ptx")
                    nc.tensor.matmul(ptx, lhsT=state[head][:, ch, :],
                                     rhs=ident16, start=True, stop=True)
                    nc.vector.tensor_copy(
                        out=ctxT_all[hd0:hd0 + HD, c0:c0 + 128], in_=ptx)

            def oproj_item(mac, mc):
                c0 = mac * MAC + mc * 128
                ot = opool.tile([128, DIM], dt.float16, tag="o")
                for e in range(DIM // 512):
                    po = fine.tile([128, 512], dt.float32, tag="f", name="po")
                    nc.tensor.matmul(po, lhsT=ctxT_all[:, c0:c0 + 128],
                                     rhs=woT[:, e * 512:(e + 1) * 512],
                                     start=True, stop=True)
                    if e == 0:
                        nc.scalar.copy(out=ot[:, e * 512:(e + 1) * 512],
                                       in_=po)
                    else:
                        nc.vector.tensor_copy(
                            out=ot[:, e * 512:(e + 1) * 512], in_=po)
                nc.sync.dma_start(out=out_d.ap()[c0:c0 + 128, :], in_=ot)

            def finish_items(mac, cus):
                state = {}
                items = []
                for head in range(HPC):
                    items.append(
                        lambda h=head: norm_stage1(h, cus[h], state))
                for ch in range(MAC // 128):
                    items.append(lambda m=mac, c=ch: norm_stage2(m, c, state))
                    items.append(lambda m=mac, c=ch: oproj_item(m, c))
                return items

            pending = []
            for mac in range(S // MAC):
                m0 = mac * MAC
                pcu = [cpool.tile([HD + 1, MAC], dt.float32, tag="pc",
                                  name=f"pcu{i}") for i in range(HPC)]
                wq = []   # (t, head, w) exp outputs awaiting their ctx matmul
                for t in range(TC):
                    for head in range(HPC):
                        hd0 = head * HD
                        ps = spool[head].tile([128, MAC], dt.float32, tag="s")
                        # K=64: the two heads run on disjoint 64-row PE
                        # tiles concurrently.
                        nc.tensor.matmul(
                            ps,
                            lhsT=kT_all[hd0:hd0 + HD, t * 128:(t + 1) * 128],
                            rhs=qT_all[hd0:hd0 + HD, m0:m0 + MAC],
                            start=True, stop=True,
                            tile_position=(hd0, 0))
                        w = wpool.tile([128, MAC], dt.float16, tag="w")
                        # head0 -> ScalarE; head1 -> VectorE poly (1 in 8
                        # back on ScalarE to balance)
                        if head == 1 and t % 8 != 7:
                            nc.vector._custom_dve(
                                exp2_op, out=w[:], in0=ps[:], in1=b4_t[:],
                                s0=EXP2_B[0], s1=EXP2_B[1], imm2=EXP2_B[2])
                        else:
                            nc.scalar.activation(out=w, in_=ps,
                                                 func=AF.Exp, scale=LN2)
                        wq.append((t, head, w))
                    # issue ctx matmuls one t-iteration behind the exps so
                    # the PE never stalls on activation latency
                    while len(wq) > 2 * HPC:
                        wt, wh, ww = wq.pop(0)
                        nc.tensor.matmul(
                            pcu[wh], lhsT=v_all[:, wt, wh, :], rhs=ww,
                            start=(wt == 0), stop=(wt == TC - 1),
                            skip_group_check=True)
                    if pending and t >= 12 and t % 2 == 0:
                        pending.pop(0)()
                for wt, wh, ww in wq:
                    nc.tensor.matmul(
                        pcu[wh], lhsT=v_all[:, wt, wh, :], rhs=ww,
                        start=(wt == 0), stop=(wt == TC - 1),
                        skip_group_check=True)
                # evacuate the accumulated ctx to SBUF (frees psum quickly);
                # normalize + out-proj run interleaved in the NEXT mac's loop
                for item in pending:   # leftovers (shouldn't happen)
                    item()
                cus = []
                for head in range(HPC):
                    cu = upool.tile([HD + 1, MAC], dt.float16, tag="cu")
                    nc.scalar.copy(out=cu, in_=pcu[head])
                    cus.append(cu)
                pending = finish_items(mac, cus)
            for item in pending:
                item()

    nc.compile()
    return nc


def kernel(**inputs):
    global LAST_RESULT
    from concourse.bass_utils import run_bass_kernel_spmd

    x = np.asarray(inputs["x"], dtype=np.float32).reshape(S, DIM)
    ln_g = np.asarray(inputs["ln_g"], dtype=np.float32)
    ln_b = np.asarray(inputs["ln_b"], dtype=np.float32)
    Wq = np.asarray(inputs["Wq"], dtype=np.float32)
    Wk = np.asarray(inputs["Wk"], dtype=np.float32)
    Wv = np.asarray(inputs["Wv"], dtype=np.float32)
    Wo = np.asarray(inputs["Wo"], dtype=np.float32)
    bq = np.asarray(inputs["bq"], dtype=np.float32)
    bk = np.asarray(inputs["bk"], dtype=np.float32)
    bv = np.asarray(inputs["bv"], dtype=np.float32)
    bo = np.asarray(inputs["bo"], dtype=np.float32)

    if "nc" not in _CACHE:
        _CACHE["nc"] = _build()
    nc = _CACHE["nc"]

    in_maps = []
    for c in range(N_CORES):
        sl = slice(c * DC, (c + 1) * DC)
        in_maps.append({
            "x": x,
            "wqT": np.ascontiguousarray(
                (Wq[sl] * ln_g[None, :]).T * QSCALE).astype(np.float16),
            "wkT": np.ascontiguousarray((Wk[sl] * ln_g[None, :]).T).astype(np.float16),
            "wvT": np.ascontiguousarray((Wv[sl] * ln_g[None, :]).T).astype(np.float16),
            "woT": np.ascontiguousarray(Wo[:, sl].T).astype(np.float16),
            "bq": (bq[sl] + Wq[sl] @ ln_b) * QSCALE,
            "bk": bk[sl] + Wk[sl] @ ln_b,
            "bv": bv[sl] + Wv[sl] @ ln_b,
        })

    res = run_bass_kernel_spmd(nc, in_maps, list(range(N_CORES)))
    LAST_RESULT = res

    acc = res.results[0]["out"].astype(np.float32)
    for c in range(1, N_CORES):
        acc = acc + res.results[c]["out"].astype(np.float32)
    acc += bo[None, :]
    return acc.reshape(B, S, DIM)


# revision 34
# speedup vs baseline: 1.1139x; 1.0083x over previous
"""Trainium2 Bass kernel: LayerNorm -> MHA(16 heads, S=4096, D=1024) -> out-proj.

Sharding: tensor-parallel over heads. 8 cores x 2 heads each.
Each core computes LN(x) (replicated), q/k/v for its 2 heads (columns of
Wq/Wk/Wv), attention for those heads, and a partial output projection
(its 128 rows of Wo.T). Host sums the 8 partials and adds bo.

v2: all matmul operands fp16 (PE runs 16-bit at 2.4 GHz vs 1.2 GHz for
fp32r), out-projection folded into phase 2 per m-macro so its compute and
output DMA hide under the softmax-exp (ScalarE) shadow, fp16 output
partials (halves output DMA).

Per-core layout (core c, heads 2c, 2c+1; d-slice = [128c, 128c+128)):
  phase 1: LN in [m,d] tiles -> PE-transpose -> hT [d,m] (fp16)
           qT/kT = WT.T @ hT   [128n(2 heads' dims), 4096m] fp16
           v     = transpose(vT) -> [t, (tc,head,65)] fp16 (ones col for sums)
  phase 2: scoresT[t,m] = kT.T @ qT per head (K=64)  -> exp (ACT, scale=1/32)
           ctx[m,d]    += w.T @ v_aug (fp16, accumulates sums in col 64)
           normalize by 1/sums (per-partition), transpose -> ctxT [d,m] fp16
           then per-mac: partial out = ctxT.T @ WoT -> DRAM (fp16)

LN gain g is folded into Wq/Wk/Wv columns host-side; LN bias b_ln is folded
into bq/bk/bv.  bo is added host-side after the cross-core reduction.
"""

import math
from contextlib import ExitStack

import numpy as np

B, S, DIM, H = 1, 4096, 1024, 16
HD = DIM // H            # 64
N_CORES = 8
HPC = H // N_CORES       # 2 heads per core
DC = HPC * HD            # 128 dims per core
MB = 512                 # phase-1 m-block
N_MB = S // MB           # 8
TC = S // 128            # 32 t-chunks
MAC = 512                # phase-2 m-macro width
SCALE = 1.0 / math.sqrt(DIM)
QSCALE = math.log2(math.e) / math.sqrt(DIM)   # folded into Wq/bq host-side

_CACHE = {}
LAST_RESULT = None       # BassKernelResults of the most recent run (for test.py)

# exp(s/sqrt(DIM)) = 2^t with t = s*log2(e)/sqrt(DIM); the q-side weights are
# pre-scaled by log2(e)/sqrt(DIM) host-side so the scores matmul emits t
# directly.  ACT tiles compute 2^t = e^(t*ln2); DVE tiles evaluate a
# minimax deg-4 polynomial p(t) ~ 2^t on [-1.6, 1.6] (max rel err 1.0e-3)
# with p(0)=1 hardwired via the One constant: 8 ALU stages, exactly the
# custom-DVE budget.  Sharing the exp work between ScalarE and VectorE is
# what lets the Tensor engine stay the bottleneck (and stay HAM-warm).
EXP2_B = (0.69178671, 0.24190469, 0.05912525, 0.00915199)   # b1..b4
DVE_FRAC = 4             # every 4th (t,head) tile goes to the DVE
LN2 = math.log(2.0)


def _register_exp2_op():
    """Add the EXP2_P4_ANT custom-DVE op to the concourse registry.

    out = (((b4*t + b3)*t + b2)*t + b1)*t + 1  with b1..b3 via s0/s1/imm2
    and b4 spilled through in1 (C3 slot).  The uops_sha is computed here
    (self-consistent by construction) since the registry pins it."""
    import concourse.dve_ops as dom
    for o in dom.OPS:
        if o.name == "EXP2_P4_ANT":
            return o
    from concourse.dve_spec import (Spec, Src0, C0, C1, C2, C3, One,
                                    _spill_c3_to_src1, lower)
    from concourse.dve_uop import DveOpSpec

    body = _spill_c3_to_src1(
        ((((C3 * Src0) + C2) * Src0 + C1) * Src0 + C0) * Src0 + One)

    def _ref(in0, in1, c0, c1, c2):
        b4 = np.asarray(in1, np.float32).reshape(-1, 1)
        t = in0.astype(np.float32)
        return ((((b4 * t) + c2) * t + c1) * t + c0) * t + 1.0

    spec = Spec(body=body, reference=_ref)
    row = dom._CUSTOM_DVE_ROW_BASE + len(dom.OPS)
    shas = {}
    for ver in ("v3", "v4"):
        s = DveOpSpec(name="EXP2_P4_ANT", opcode=row,
                      uops=lower(spec, ver=ver), rd1_en=True)
        shas[ver] = s.sha(ver)
    op = dom.DveOp("EXP2_P4_ANT", spec, subdim=False, uops_sha=shas)
    dom.OPS.append(op)
    dom._SUB_OPCODE_FOR_NAME["EXP2_P4_ANT"] = row
    return op


def _build():
    import concourse.bacc as bacc
    import concourse.tile as tile
    import concourse.mybir as mybir
    from concourse.masks import make_identity

    dt = mybir.dt
    AF = mybir.ActivationFunctionType
    ALU = mybir.AluOpType

    exp2_op = _register_exp2_op()

    nc = bacc.Bacc("TRN2", target_bir_lowering=False, debug=False,
                   num_devices=N_CORES)

    x_d = nc.dram_tensor("x", [S, DIM], dt.float32, kind="ExternalInput")
    wqT_d = nc.dram_tensor("wqT", [DIM, DC], dt.float16, kind="ExternalInput")
    wkT_d = nc.dram_tensor("wkT", [DIM, DC], dt.float16, kind="ExternalInput")
    wvT_d = nc.dram_tensor("wvT", [DIM, DC], dt.float16, kind="ExternalInput")
    woT_d = nc.dram_tensor("woT", [DC, DIM], dt.float16, kind="ExternalInput")
    bq_d = nc.dram_tensor("bq", [DC], dt.float32, kind="ExternalInput")
    bk_d = nc.dram_tensor("bk", [DC], dt.float32, kind="ExternalInput")
    bv_d = nc.dram_tensor("bv", [DC], dt.float32, kind="ExternalInput")
    out_d = nc.dram_tensor("out", [S, DIM], dt.float16, kind="ExternalOutput")

    with tile.TileContext(nc) as tc, ExitStack() as top:
        persist = top.enter_context(tc.tile_pool(name="persist", bufs=1))

        # --- persistent tiles ---
        ident = persist.tile([128, 128], dt.float32)
        make_identity(nc, ident)
        ident16 = persist.tile([128, 128], dt.float16)
        nc.vector.tensor_copy(out=ident16, in_=ident)

        eps_t = persist.tile([128, 1], dt.float32)
        nc.vector.memset(eps_t, 1e-5)
        b4_t = persist.tile([128, 1], dt.float32)
        nc.vector.memset(b4_t, EXP2_B[3])

        wT = {}
        for name, d in (("q", wqT_d), ("k", wkT_d), ("v", wvT_d)):
            t = persist.tile([128, DIM // 128, DC], dt.float16,
                             tag=f"w{name}T", name=f"w{name}T")
            nc.scalar.dma_start(out=t, in_=d.ap().rearrange(
                "(c p) n -> p c n", p=128))
            wT[name] = t
        woT = persist.tile([DC, DIM], dt.float16)
        nc.scalar.dma_start(out=woT, in_=woT_d.ap())
        bias = {}
        for name, d in (("q", bq_d), ("k", bk_d), ("v", bv_d)):
            t = persist.tile([DC, 1], dt.float32, tag=f"b{name}",
                             name=f"b{name}")
            nc.scalar.dma_start(out=t, in_=d.ap()[:, None])
            bias[name] = t

        qT_all = persist.tile([DC, S], dt.float16)
        kT_all = persist.tile([DC, S], dt.float16)
        # v with an appended ones-column per head: [t-part, tc, head, HD+1]
        v_all = persist.tile([128, TC, HPC, HD + 1], dt.float16)
        nc.vector.memset(v_all, 1.0)
        ctxT_all = persist.tile([DC, S], dt.float16)

        # ---------------- phase 1: LN + QKV projections ----------------
        with ExitStack() as p1:
            xpool = p1.enter_context(tc.tile_pool(name="xp", bufs=10))
            hpool = p1.enter_context(tc.tile_pool(name="hp", bufs=5))
            hTpool = p1.enter_context(tc.tile_pool(name="hTp", bufs=2))
            stat = p1.enter_context(tc.tile_pool(name="stat", bufs=8))
            vsb = p1.enter_context(tc.tile_pool(name="vsb", bufs=2))
            ps_t = p1.enter_context(tc.tile_pool(name="ps_t", bufs=2, space="PSUM"))
            ps_p = p1.enter_context(tc.tile_pool(name="ps_p", bufs=3, space="PSUM"))
            ps_v = p1.enter_context(tc.tile_pool(name="ps_v", bufs=1, space="PSUM"))

            for mb in range(N_MB):
                hs = []
                for j in range(MB // 128):
                    r0 = mb * MB + j * 128
                    xt = xpool.tile([128, DIM], dt.float32, tag="x")
                    nc.sync.dma_start(out=xt, in_=x_d.ap()[r0:r0 + 128, :])
                    # LayerNorm stats
                    st = stat.tile([128, 2, nc.vector.BN_STATS_DIM],
                                   dt.float32, tag="st")
                    xg = xt[:].rearrange("p (s f) -> p s f", s=2)
                    for sg in range(2):
                        nc.vector.bn_stats(out=st[:, sg, :], in_=xg[:, sg, :])
                    mv = stat.tile([128, 2], dt.float32, tag="mv")
                    nc.vector.bn_aggr(out=mv, in_=st)
                    std = stat.tile([128, 1], dt.float32, tag="sd")
                    nc.scalar.activation(out=std, in_=mv[:, 1:2], func=AF.Sqrt,
                                         bias=eps_t, scale=1.0)
                    rstd = stat.tile([128, 1], dt.float32, tag="rs")
                    nc.vector.reciprocal(out=rstd, in_=std)
                    ht = hpool.tile([128, DIM], dt.float16, tag="h")
                    nc.vector.tensor_scalar(out=ht, in0=xt, scalar1=mv[:, 0:1],
                                            scalar2=rstd, op0=ALU.subtract,
                                            op1=ALU.mult)
                    hs.append(ht)

                # transpose h -> hT  [128d, dc, 512m]
                hT = hTpool.tile([128, DIM // 128, MB], dt.float16, tag="hT")
                for dc in range(DIM // 128):
                    pt = ps_t.tile([128, MB], dt.float32, tag="pt")
                    for j in range(MB // 128):
                        nc.tensor.matmul(
                            pt[:, j * 128:(j + 1) * 128],
                            lhsT=hs[j][:, dc * 128:(dc + 1) * 128],
                            rhs=ident16, start=True, stop=True)
                    nc.scalar.copy(out=hT[:, dc, :], in_=pt)

                # q/k/v projections for this m-block: [128n, 512m]
                for name, dest in (("q", qT_all), ("k", kT_all), ("v", None)):
                    pp = ps_p.tile([128, MB], dt.float32, tag="pp")
                    for dc in range(DIM // 128):
                        nc.tensor.matmul(pp, lhsT=wT[name][:, dc, :],
                                         rhs=hT[:, dc, :],
                                         start=(dc == 0), stop=(dc == 7))
                    if dest is not None:
                        nc.scalar.activation(
                            out=dest[:, mb * MB:(mb + 1) * MB], in_=pp,
                            func=AF.Identity, bias=bias[name], scale=1.0)
                    else:
                        vT = vsb.tile([128, MB], dt.float16, tag="vT")
                        nc.vector.tensor_scalar(
                            out=vT, in0=pp, scalar1=bias[name], scalar2=None,
                            op0=ALU.add)
                        pv = ps_v.tile([128, MB], dt.float32, tag="pv")
                        for j in range(MB // 128):
                            nc.tensor.matmul(
                                pv[:, j * 128:(j + 1) * 128],
                                lhsT=vT[:, j * 128:(j + 1) * 128],
                                rhs=ident16, start=True, stop=True)
                        for j in range(MB // 128):
                            tc_j = mb * (MB // 128) + j
                            src = pv[:, j * 128:(j + 1) * 128].rearrange(
                                "p (h e) -> p h e", h=HPC)
                            nc.scalar.copy(
                                out=v_all[:, tc_j, :, 0:HD], in_=src)

        # ---------------- phase 2: attention + out-projection ----------------
        # ctxT-direct: lhsT = v_aug [128t, 65] (stationary), rhs = w [128t, m]
        # -> ctxT_u [65, m] accumulated in psum (row 64 = softmax sums).
        # Scores pipeline at [128,512] half-tile granularity (3 psum bufs);
        # exp halves are split ~60/40 between ScalarE and the custom
        # VectorE polynomial so neither engine gates the Tensor engine.
        # The previous mac's normalize + out-proj is INTERLEAVED into the
        # current mac's t-loop: keeps real matmuls flowing so the PE HAM
        # clock-gate never re-throttles on a transpose-only stretch.
        with ExitStack() as p2:
            sp0 = p2.enter_context(tc.tile_pool(name="sp0", bufs=2, space="PSUM"))
            sp1 = p2.enter_context(tc.tile_pool(name="sp1", bufs=2, space="PSUM"))
            spool = [sp0, sp1]
            cpool = p2.enter_context(tc.tile_pool(name="cp", bufs=2, space="PSUM"))
            fine = p2.enter_context(tc.tile_pool(name="fine", bufs=2, space="PSUM"))
            wpool = p2.enter_context(tc.tile_pool(name="wp", bufs=8))
            upool = p2.enter_context(tc.tile_pool(name="up", bufs=4))
            npool = p2.enter_context(tc.tile_pool(name="np", bufs=6))
            opool = p2.enter_context(tc.tile_pool(name="op", bufs=3))

            NCH = MAC // 128

            def norm_stage1(head, cu, state):
                # transpose the whole mac's ctx+sums for one head, batch the
                # reciprocal and the 1/sums multiply (stride 66 keeps each
                # matmul's psum write 8-byte aligned)
                ptn4 = fine.tile([128, NCH, HD + 2], dt.float32, tag="f",
                                 name="ptn4")
                for ch in range(NCH):
                    nc.tensor.matmul(
                        ptn4[:, ch, 0:HD + 1],
                        lhsT=cu[:, ch * 128:(ch + 1) * 128],
                        rhs=ident16[0:HD + 1, 0:HD + 1],
                        start=True, stop=True)
                rec4 = npool.tile([128, NCH], dt.float32, tag="rec")
                nc.vector.reciprocal(out=rec4, in_=ptn4[:, :, HD])
                cn4 = npool.tile([128, NCH, HD], dt.float16, tag="cn")
                nc.vector.tensor_tensor(
                    out=cn4, in0=ptn4[:, :, 0:HD],
                    in1=rec4[:].unsqueeze(-1).broadcast_to([128, NCH, HD]),
                    op=ALU.mult)
                state[head] = cn4

            def norm_stage2(mac, ch, state):
                c0 = mac * MAC + ch * 128
                for head in range(HPC):
                    hd0 = head * HD
                    ptx = fine.tile([HD, 128], dt.float32, tag="f", name="ptx")
                    nc.tensor.matmul(ptx, lhsT=state[head][:, ch, :],
                                     rhs=ident16, start=True, stop=True)
                    nc.vector.tensor_copy(
                        out=ctxT_all[hd0:hd0 + HD, c0:c0 + 128], in_=ptx)

            def oproj_item(mac, mc):
                c0 = mac * MAC + mc * 128
                ot = opool.tile([128, DIM], dt.float16, tag="o")
                for e in range(DIM // 512):
                    po = fine.tile([128, 512], dt.float32, tag="f", name="po")
                    nc.tensor.matmul(po, lhsT=ctxT_all[:, c0:c0 + 128],
                                     rhs=woT[:, e * 512:(e + 1) * 512],
                                     start=True, stop=True)
                    if e == 0:
                        nc.scalar.copy(out=ot[:, e * 512:(e + 1) * 512],
                                       in_=po)
                    else:
                        nc.vector.tensor_copy(
                            out=ot[:, e * 512:(e + 1) * 512], in_=po)
                nc.sync.dma_start(out=out_d.ap()[c0:c0 + 128, :], in_=ot)

            def finish_items(mac, cus):
                state = {}
                items = []
                for head in range(HPC):
                    items.append(
                        lambda h=head: norm_stage1(h, cus[h], state))
                for ch in range(MAC // 128):
                    items.append(lambda m=mac, c=ch: norm_stage2(m, c, state))
                    items.append(lambda m=mac, c=ch: oproj_item(m, c))
                return items

            pending = []
            for mac in range(S // MAC):
                m0 = mac * MAC
                pcu = [cpool.tile([HD + 1, MAC], dt.float32, tag="pc",
                                  name=f"pcu{i}") for i in range(HPC)]
                wq = []   # (t, head, w) exp outputs awaiting their ctx matmul
                for t in range(TC):
                    for head in range(HPC):
                        hd0 = head * HD
                        ps = spool[head].tile([128, MAC], dt.float32, tag="s")
                        # K=64: the two heads run on disjoint 64-row PE
                        # tiles concurrently.
                        nc.tensor.matmul(
                            ps,
                            lhsT=kT_all[hd0:hd0 + HD, t * 128:(t + 1) * 128],
                            rhs=qT_all[hd0:hd0 + HD, m0:m0 + MAC],
                            start=True, stop=True,
                            tile_position=(hd0, 0))
                        w = wpool.tile([128, MAC], dt.float16, tag="w")
                        # head0 -> ScalarE; head1 -> VectorE poly (1 in 8
                        # back on ScalarE to balance)
                        if head == 1 and t % 8 != 7:
                            nc.vector._custom_dve(
                                exp2_op, out=w[:], in0=ps[:], in1=b4_t[:],
                                s0=EXP2_B[0], s1=EXP2_B[1], imm2=EXP2_B[2])
                        else:
                            nc.scalar.activation(out=w, in_=ps,
                                                 func=AF.Exp, scale=LN2)
                        wq.append((t, head, w))
                    # issue ctx matmuls one t-iteration behind the exps so
                    # the PE never stalls on activation latency
                    while len(wq) > 2 * HPC:
                        wt, wh, ww = wq.pop(0)
                        nc.tensor.matmul(
                            pcu[wh], lhsT=v_all[:, wt, wh, :], rhs=ww,
                            start=(wt == 0), stop=(wt == TC - 1),
                            skip_group_check=True)
                    if pending and t >= 12 and t % 2 == 0:
                        pending.pop(0)()
                for wt, wh, ww in wq:
                    nc.tensor.matmul(
                        pcu[wh], lhsT=v_all[:, wt, wh, :], rhs=ww,
                        start=(wt == 0), stop=(wt == TC - 1),
                        skip_group_check=True)
                # evacuate the accumulated ctx to SBUF (frees psum quickly);
                # normalize + out-proj run interleaved in the NEXT mac's loop
                for item in pending:   # leftovers (shouldn't happen)
                    item()
                cus = []
                for head in range(HPC):
                    cu = upool.tile([HD + 1, MAC], dt.float16, tag="cu")
                    nc.scalar.copy(out=cu, in_=pcu[head])
                    cus.append(cu)
                pending = finish_items(mac, cus)
            for item in pending:
                item()

    nc.compile()
    return nc


def kernel(**inputs):
    global LAST_RESULT
    from concourse.bass_utils import run_bass_kernel_spmd

    x = np.asarray(inputs["x"], dtype=np.float32).reshape(S, DIM)
    ln_g = np.asarray(inputs["ln_g"], dtype=np.float32)
    ln_b = np.asarray(inputs["ln_b"], dtype=np.float32)
    Wq = np.asarray(inputs["Wq"], dtype=np.float32)
    Wk = np.asarray(inputs["Wk"], dtype=np.float32)
    Wv = np.asarray(inputs["Wv"], dtype=np.float32)
    Wo = np.asarray(inputs["Wo"], dtype=np.float32)
    bq = np.asarray(inputs["bq"], dtype=np.float32)
    bk = np.asarray(inputs["bk"], dtype=np.float32)
    bv = np.asarray(inputs["bv"], dtype=np.float32)
    bo = np.asarray(inputs["bo"], dtype=np.float32)

    if "nc" not in _CACHE:
        _CACHE["nc"] = _build()
    nc = _CACHE["nc"]

    in_maps = []
    for c in range(N_CORES):
        sl = slice(c * DC, (c + 1) * DC)
        in_maps.append({
            "x": x,
            "wqT": np.ascontiguousarray(
                (Wq[sl] * ln_g[None, :]).T * QSCALE).astype(np.float16),
            "wkT": np.ascontiguousarray((Wk[sl] * ln_g[None, :]).T).astype(np.float16),
            "wvT": np.ascontiguousarray((Wv[sl] * ln_g[None, :]).T).astype(np.float16),
            "woT": np.ascontiguousarray(Wo[:, sl].T).astype(np.float16),
            "bq": (bq[sl] + Wq[sl] @ ln_b) * QSCALE,
            "bk": bk[sl] + Wk[sl] @ ln_b,
            "bv": bv[sl] + Wv[sl] @ ln_b,
        })

    res = run_bass_kernel_spmd(nc, in_maps, list(range(N_CORES)))
    LAST_RESULT = res

    acc = res.results[0]["out"].astype(np.float32)
    for c in range(1, N_CORES):
        acc = acc + res.results[c]["out"].astype(np.float32)
    acc += bo[None, :]
    return acc.reshape(B, S, DIM)
